# revision 1
# baseline (speedup 1.0000x reference)
import sys

sys.path.insert(0, "/opt/trn_rl_repo")
import numpy as np

# --- Problem geometry (hardcoded from the nn_DifferentiableBackprojection spec) ---
B, C, A, V, U = 1, 8, 120, 128, 128
NZ, NY, NX = 96, 96, 96
DSO = 1000.0
DSD = 1500.0
DU = DV = 1.0
DVOX = 0.8
NYX = NY * NX  # 9216
KB = 8  # v-band taps per z (covers max band width)
N_CORES = 8

_prog_cache = {}


def _geom_jax(angles):
    """iu [A, NYX], iv [A, NZ, NYX] in fp32, computed with jax on CPU using the
    exact op sequence of the reference (so floor() knife-edges agree)."""
    import jax
    import jax.numpy as jnp

    cpu = jax.devices("cpu")[0]

    @jax.jit
    def geom(angles):
        z = (jnp.arange(NZ, dtype=jnp.float32) - (NZ - 1) / 2.0) * DVOX
        y = (jnp.arange(NY, dtype=jnp.float32) - (NY - 1) / 2.0) * DVOX
        x = (jnp.arange(NX, dtype=jnp.float32) - (NX - 1) / 2.0) * DVOX
        zg, yg, xg = z[:, None, None], y[None, :, None], x[None, None, :]

        def one(ang):
            c, s = jnp.cos(ang), jnp.sin(ang)
            xr = xg * c + yg * s
            yr = -xg * s + yg * c
            dist = DSO - xr
            mag = DSD / dist
            iu = jnp.broadcast_to(
                yr * mag / DU + (U - 1) / 2.0, (NZ, NY, NX)
            ).reshape(NZ * NY * NX)[: NY * NX]
            iv = (zg * mag / DV + (V - 1) / 2.0).reshape(NZ, NY * NX)
            w = jnp.broadcast_to(mag * mag, (NZ, NY, NX)).reshape(NZ * NY * NX)[
                : NY * NX
            ]
            return iu, iv, w

        return jax.vmap(one)(angles)

    with jax.default_device(cpu):
        iu, iv, w = geom(jnp.asarray(angles, dtype=jnp.float32))
    return np.asarray(iu), np.asarray(iv), np.asarray(w)


def _host_tables(angles):
    """Per-angle geometry tables, replicating reference.py ops in float32.

    Returns:
      gu:  [A, U, NYX] f16    u-interp hat weights * distance weight * valid
      h:   [A, KB, NZ, NYX] f16   v-interp hat weights
      rows:[A, KB, NZ] int64  sinogram v-row index per tap (clipped)
    """
    f32 = np.float32
    iu_all, iv_all, w_all = _geom_jax(angles)

    gu = np.zeros((A, U, NYX), np.float16)
    h = np.zeros((A, KB, NZ, NYX), np.float16)
    rows = np.zeros((A, KB, NZ), np.int64)

    uu = np.arange(U, dtype=f32)[:, None]  # [U, 1]

    for a in range(A):
        iu = iu_all[a]
        iv = iv_all[a]
        assert iv.min() >= 0.0 and iv.max() <= V - 1, "iv out of range"
        valid = (iu >= 0) & (iu <= U - 1)
        w = w_all[a] * valid.astype(f32)

        # u hats: relu(1 - |u - iu|) * w  == exact bilinear u-weights (valid voxels)
        gu[a] = (
            np.maximum(f32(0.0), f32(1.0) - np.abs(uu - iu[None, :])) * w[None, :]
        ).astype(np.float16)

        v0 = np.floor(iv).astype(np.int64)
        b = v0.min(axis=1)  # [NZ]
        assert int((v0.max(axis=1) - b).max()) <= KB - 2, "band too wide"
        for k in range(KB):
            j = b + k  # [NZ]
            h[a, k] = np.maximum(
                f32(0.0), f32(1.0) - np.abs(iv - j[:, None].astype(f32))
            ).astype(np.float16)
            rows[a, k] = np.clip(j, 0, V - 1)
    return gu, h, rows


def _build_program():
    if "nc" in _prog_cache:
        return _prog_cache["nc"]
    import concourse.bass as bass
    import concourse.tile as tile
    from concourse import mybir, bacc

    FREE = KB * NZ + NYX  # 768 + 9216
    CH = 2048  # mult/add chunk (4 PSUM banks)

    nc = bacc.Bacc("TRN2", target_bir_lowering=False, debug=False)
    segu_d = nc.dram_tensor(
        "segu", (A, U, FREE), mybir.dt.float16, kind="ExternalInput"
    )
    h_d = nc.dram_tensor("h", (A, KB, NZ, NYX), mybir.dt.float16, kind="ExternalInput")
    out_d = nc.dram_tensor("out", (NZ, NYX), mybir.dt.float32, kind="ExternalOutput")

    with tile.TileContext(nc) as tc:
        with (
            tc.tile_pool(name="persist", bufs=1) as pp,
            tc.tile_pool(name="io", bufs=2) as io,
            tc.tile_pool(name="work", bufs=2) as wk,
            tc.tile_pool(name="ps", bufs=2, space=bass.MemorySpace.PSUM) as ps,
        ):
            acc = pp.tile([NZ, NYX], mybir.dt.float32)
            nc.vector.memset(acc[:], 0.0)

            for a in range(A):
                segu = io.tile([U, FREE], mybir.dt.float16, tag="segu")
                nc.gpsimd.dma_start(segu[:], segu_d.ap()[a])
                for k in range(KB):
                    hk = io.tile([NZ, NYX], mybir.dt.float16, tag="hk")
                    nc.gpsimd.dma_start(hk[:], h_d.ap()[a, k])
                    for n0 in range(0, NYX, CH):
                        n = min(CH, NYX - n0)
                        te = ps.tile([NZ, CH], mybir.dt.float32, tag="te")
                        for j in range(0, n, 512):
                            nc.tensor.matmul(
                                te[:, j : j + 512],
                                segu[:, k * NZ : (k + 1) * NZ],
                                segu[:, KB * NZ + n0 + j : KB * NZ + n0 + j + 512],
                                start=True,
                                stop=True,
                            )
                        m = wk.tile([NZ, CH], mybir.dt.float32, tag="m")
                        nc.vector.tensor_mul(m[:, :n], te[:, :n], hk[:, n0 : n0 + n])
                        nc.vector.tensor_add(
                            acc[:, n0 : n0 + n], acc[:, n0 : n0 + n], m[:, :n]
                        )
            nc.sync.dma_start(out_d.ap(), acc[:])
    nc.compile()
    _prog_cache["nc"] = nc
    return nc


def _install_ntff_shim():
    """Provide antenv.axon_hooks (missing in this image) so trace=True works."""
    import types, importlib

    try:
        from antenv.axon_hooks import get_axon_ntff_profile_hook  # noqa: F401

        return True
    except ImportError:
        pass
    try:
        import antenv

        mod = types.ModuleType("antenv.axon_hooks")
        mod._hook = None

        def set_axon_ntff_profile_hook(h):
            mod._hook = h

        def get_axon_ntff_profile_hook():
            return mod._hook

        mod.set_axon_ntff_profile_hook = set_axon_ntff_profile_hook
        mod.get_axon_ntff_profile_hook = get_axon_ntff_profile_hook
        sys.modules["antenv.axon_hooks"] = mod
        antenv.axon_hooks = mod
        if "/root/.axon_site" not in sys.path:
            sys.path.insert(0, "/root/.axon_site")
        boot = importlib.import_module("trn_agent_boot.trn_boot")
        hook = boot._ntff_profile_via_ctypes("/opt/axon/libaxon_pjrt.so")
        if hook is None:
            return False
        mod._hook = hook
        return True
    except Exception as e:  # pragma: no cover
        print(f"ntff shim failed: {e}")
        return False


def kernel(sinogram, angles):
    import os
    from concourse.bass_utils import run_bass_kernel_spmd

    sinogram = np.asarray(sinogram)
    angles = np.asarray(angles)
    in_dtype = sinogram.dtype
    gu, h, rows = _host_tables(angles)

    sino = sinogram.reshape(C, A, V, U).astype(np.float32)
    ai = np.arange(A)[:, None, None]
    in_maps = []
    for c in range(C):
        se = sino[c][ai, rows]  # [A, KB, NZ, U]
        se_t = np.ascontiguousarray(np.transpose(se, (0, 3, 1, 2)))  # [A, U, KB, NZ]
        segu = np.concatenate(
            [se_t.reshape(A, U, KB * NZ).astype(np.float16), gu], axis=2
        )  # [A, U, KB*NZ + NYX]
        in_maps.append({"segu": np.ascontiguousarray(segu), "h": h})

    nc = _build_program()
    trace = bool(os.environ.get("BP_TRACE")) and _install_ntff_shim()
    res = run_bass_kernel_spmd(nc, in_maps, list(range(N_CORES)), trace=trace)
    _prog_cache["last_results"] = res
    vols = np.stack(
        [res.results[i]["out"].reshape(NZ, NY, NX) for i in range(N_CORES)]
    )
    return vols.reshape(B, C, NZ, NY, NX).astype(in_dtype, copy=False)



# revision 3
# speedup vs baseline: 3.4903x; 3.4903x over previous
import sys

sys.path.insert(0, "/opt/trn_rl_repo")
import numpy as np

# --- Problem geometry (hardcoded from the nn_DifferentiableBackprojection spec) ---
B, C, A, V, U = 1, 8, 120, 128, 128
NZ, NY, NX = 96, 96, 96
DSO, DSD, DU, DV, DVOX = 1000.0, 1500.0, 1.0, 1.0, 0.8
NYX = NY * NX
TY = TX = 3
TH, TW = NY // TY, NX // TX
FD = TH * TW  # 1024 cols per tile
NT = TY * TX
MARGIN = 1e-3
WIN = 8       # groups per fp16-partial flush window
N_CORES = 8

_prog_cache = {}


def _geom_jax(angles):
    """Bit-exact (vs reference) iu [A,NYX], mag [A,NYX] using jax f32 on CPU."""
    import jax
    import jax.numpy as jnp

    cpu = jax.devices("cpu")[0]

    @jax.jit
    def geom(angles):
        y = (jnp.arange(NY, dtype=jnp.float32) - (NY - 1) / 2.0) * DVOX
        x = (jnp.arange(NX, dtype=jnp.float32) - (NX - 1) / 2.0) * DVOX
        yg, xg = y[:, None], x[None, :]

        def one(ang):
            c, s = jnp.cos(ang), jnp.sin(ang)
            xr = xg * c + yg * s
            yr = -xg * s + yg * c
            mag = DSD / (DSO - xr)
            iu = (yr * mag / DU + (U - 1) / 2.0).reshape(NYX)
            return iu, mag.reshape(NYX)

        return jax.vmap(one)(angles)

    with jax.default_device(cpu):
        iu, mag = geom(jnp.asarray(angles, dtype=jnp.float32))
    return np.asarray(iu), np.asarray(mag)


def _find_groups(angles):
    """Group angles by mirror symmetry: [(canonical_idx, [(aidx, flip_j, urev)])].

    flip_j: 0=identity, 1=x-flip, 2=xy-flip, 3=y-flip (of the output volume);
    urev: reverse sinogram u-axis for that partner. Falls back to singleton
    groups when the angle set is not the uniform symmetric one.
    """
    ang = np.asarray(angles, dtype=np.float64)
    n = len(ang)
    if not (n % 4 == 0 and np.allclose(ang, np.arange(n) * 2 * np.pi / n, atol=1e-5)):
        return [(i, [(i, 0, False)]) for i in range(n)]
    q = n // 4
    groups = []
    for a in range(q + 1):
        partners = [(a, 0, False)]
        seen = {a}
        for idx, j, urev in [((2 * q - a) % n, 1, True),
                             ((2 * q + a) % n, 2, False),
                             ((n - a) % n, 3, True)]:
            if idx not in seen:
                partners.append((idx, j, urev))
                seen.add(idx)
        groups.append((a, partners))
    allidx = sorted(i for _, ps in groups for i, _, _ in ps)
    assert allidx == list(range(n)), "angle symmetry grouping failed"
    return groups


def _tile_cols(t):
    ty, tx = t // TX, t % TX
    yy = np.arange(ty * TH, (ty + 1) * TH)
    xx = np.arange(tx * TW, (tx + 1) * TW)
    return (yy[:, None] * NX + xx[None, :]).reshape(-1)


def _build_tables(angles):
    """Per-(group,tile) geometry tables shared across channels and partners."""
    f32 = np.float32
    iu_all, mag_all = _geom_jax(angles)
    groups = _find_groups(angles)
    zg = ((np.arange(NZ, dtype=f32) - (NZ - 1) / 2.0) * DVOX).astype(f32)
    uu = np.arange(U, dtype=f32)

    units = []
    for ci, partners in groups:
        iu, mag = iu_all[ci], mag_all[ci]
        wv = (mag * mag) * ((iu >= 0) & (iu <= U - 1)).astype(f32)
        for t in range(NT):
            cols = _tile_cols(t)
            iu_t, wv_t, mag_t = iu[cols], wv[cols], mag[cols]
            gu = (np.maximum(f32(0), f32(1) - np.abs(uu[:, None] - iu_t[None, :]))
                  * wv_t[None, :]).astype(np.float16)          # [U, FD]
            iv_t = zg[:, None] * mag_t[None, :] + f32((V - 1) / 2.0)  # [NZ, FD]
            b = np.floor(iv_t.min(axis=1) - MARGIN).astype(np.int64)
            top = np.floor(iv_t.max(axis=1) + MARGIN).astype(np.int64) + 1
            T = int((top - b).max()) + 1
            jrows = b[None, :] + np.arange(T)[:, None]          # [T, NZ]
            h = np.maximum(
                f32(0), f32(1) - np.abs(iv_t[None] - jrows[:, :, None].astype(f32))
            ).astype(np.float16)                                # [T, NZ, FD]
            units.append(dict(gu=gu, h=h, rows=np.clip(jrows, 0, V - 1),
                              taps=T, partners=partners, tile=t))
    return units, groups


def _pack_tables(units):
    """Pack ragged per-unit tables into flat dram arrays + offsets."""
    nu = len(units)
    gu = np.stack([u["gu"] for u in units])                     # [NU, U, FD]
    h_off, se_off = [], []
    hcols = scols = 0
    for u in units:
        h_off.append(hcols)
        se_off.append(scols)
        hcols += u["taps"] * FD
        scols += len(u["partners"]) * u["taps"] * 96
    h = np.zeros((NZ, hcols), np.float16)
    for u, off in zip(units, h_off):
        T = u["taps"]
        # h[k][z, :] -> columns [off + k*FD : off + (k+1)*FD]
        h[:, off:off + T * FD] = np.transpose(u["h"], (1, 0, 2)).reshape(NZ, T * FD)
    return gu, h, h_off, hcols, se_off, scols


def _build_se(sino_c, units, scols):
    """Gathered sinogram rows for one channel: [U, scols] f16."""
    se = np.empty((U, scols), np.float16)
    col = 0
    for un in units:
        T, rows = un["taps"], un["rows"]
        for (aidx, j, urev) in un["partners"]:
            g = sino_c[aidx][rows.reshape(-1)]     # [T*96, U]
            if urev:
                g = g[:, ::-1]
            n = T * 96
            se[:, col:col + n] = g.T
            col += n
    assert col == scols
    return se


def _pairs(T):
    return [tuple(range(i, min(i + 2, T))) for i in range(0, T, 2)]


def _build_program(units, groups, hcols, scols, h_off, se_off):
    import concourse.bass as bass
    import concourse.tile as tile
    from concourse import mybir, bacc

    nu = len(units)
    nc = bacc.Bacc("TRN2", target_bir_lowering=False, debug=False)
    gu_d = nc.dram_tensor("gu", (nu, U, FD), mybir.dt.float16, kind="ExternalInput")
    h_d = nc.dram_tensor("h", (NZ, hcols), mybir.dt.float16, kind="ExternalInput")
    se_d = nc.dram_tensor("se", (U, scols), mybir.dt.float16, kind="ExternalInput")
    out_d = nc.dram_tensor("out", (4, NZ, NT * FD), mybir.dt.float32,
                           kind="ExternalOutput")

    ngroups = len(groups)
    with tile.TileContext(nc) as tc:
        with (
            tc.tile_pool(name="persist", bufs=1) as pp,
            tc.tile_pool(name="io", bufs=2) as io,
            tc.tile_pool(name="wk", bufs=2) as wk,
            tc.tile_pool(name="ps", bufs=2, space=bass.MemorySpace.PSUM) as ps,
        ):
            acc = []
            for j in range(4):
                a = pp.tile([NZ, NT * FD], mybir.dt.float32, name=f"acc{j}")
                nc.vector.memset(a[:], 0.0)
                acc.append(a)
            partial = [pp.tile([NZ, 2 * FD], mybir.dt.float16, name=f"part{j}")
                       for j in range(4)]

            for t in range(NT):
                fresh = [True] * 4
                for gi in range(ngroups):
                    ui = gi * NT + t
                    un = units[ui]
                    T, J = un["taps"], len(un["partners"])
                    gu_t = io.tile([U, FD], mybir.dt.float16, tag="gu")
                    nc.sync.dma_start(gu_t[:], gu_d.ap()[ui])
                    h_t = io.tile([NZ, T * FD], mybir.dt.float16, tag="h")
                    nc.sync.dma_start(h_t[:], h_d.ap()[:, h_off[ui]:h_off[ui] + T * FD])
                    se_t = io.tile([U, J * T * 96], mybir.dt.float16, tag="se")
                    nc.sync.dma_start(
                        se_t[:], se_d.ap()[:, se_off[ui]:se_off[ui] + J * T * 96])

                    for ji, (aidx, j, urev) in enumerate(un["partners"]):
                        for pair in _pairs(T):
                            pw = len(pair) * FD
                            te = ps.tile([NZ, 2 * FD], mybir.dt.float32, tag="te")
                            for pi, k in enumerate(pair):
                                sek = se_t[:, (ji * T + k) * 96:(ji * T + k + 1) * 96]
                                for h2 in range(0, FD, 512):
                                    nc.tensor.matmul(
                                        te[:, pi * FD + h2:pi * FD + h2 + 512],
                                        sek, gu_t[:, h2:h2 + 512],
                                        start=True, stop=True)
                            te16 = wk.tile([NZ, 2 * FD], mybir.dt.float16, tag="te16")
                            nc.scalar.copy(te16[:, :pw], te[:, :pw])
                            hs = h_t[:, pair[0] * FD:pair[0] * FD + pw]
                            if fresh[j]:
                                nc.vector.tensor_mul(
                                    partial[j][:, :pw], te16[:, :pw], hs)
                                if pw < 2 * FD:
                                    nc.vector.memset(partial[j][:, pw:], 0.0)
                                fresh[j] = False
                            else:
                                nc.vector.tensor_mul(te16[:, :pw], te16[:, :pw], hs)
                                nc.vector.tensor_add(
                                    partial[j][:, :pw], partial[j][:, :pw],
                                    te16[:, :pw])
                    last = gi == ngroups - 1
                    if (gi + 1) % WIN == 0 or last:
                        for j in range(4):
                            if not fresh[j]:
                                at = acc[j][:, t * FD:(t + 1) * FD]
                                nc.vector.tensor_add(at, at, partial[j][:, :FD])
                                nc.vector.tensor_add(at, at, partial[j][:, FD:])
                                fresh[j] = True
            for j in range(4):
                nc.sync.dma_start(out_d.ap()[j], acc[j][:])
    nc.compile()
    return nc


def _install_ntff_shim():
    """Provide antenv.axon_hooks (missing in this image) so trace=True works."""
    import types, importlib

    try:
        from antenv.axon_hooks import get_axon_ntff_profile_hook  # noqa: F401

        return True
    except ImportError:
        pass
    try:
        import antenv

        mod = types.ModuleType("antenv.axon_hooks")
        mod._hook = None

        def set_axon_ntff_profile_hook(h):
            mod._hook = h

        def get_axon_ntff_profile_hook():
            return mod._hook

        mod.set_axon_ntff_profile_hook = set_axon_ntff_profile_hook
        mod.get_axon_ntff_profile_hook = get_axon_ntff_profile_hook
        sys.modules["antenv.axon_hooks"] = mod
        antenv.axon_hooks = mod
        if "/root/.axon_site" not in sys.path:
            sys.path.insert(0, "/root/.axon_site")
        boot = importlib.import_module("trn_agent_boot.trn_boot")
        hook = boot._ntff_profile_via_ctypes("/opt/axon/libaxon_pjrt.so")
        if hook is None:
            return False
        mod._hook = hook
        return True
    except Exception as e:  # pragma: no cover
        print(f"ntff shim failed: {e}")
        return False


def _combine_flips(acc_f32):
    """acc [4, NZ, NT*FD] tile-major -> volume [NZ, NY, NX]."""
    full = np.empty((4, NZ, NY, NX), dtype=np.float32)
    v = acc_f32.reshape(4, NZ, TY, TX, TH, TW)
    for ty in range(TY):
        for tx in range(TX):
            full[:, :, ty * TH:(ty + 1) * TH, tx * TW:(tx + 1) * TW] = v[:, :, ty, tx]
    out = full[0]
    out += full[1][:, :, ::-1]
    out += full[2][:, ::-1, ::-1]
    out += full[3][:, ::-1, :]
    return out


def kernel(sinogram, angles):
    import os
    from concourse.bass_utils import run_bass_kernel_spmd

    sinogram = np.asarray(sinogram)
    in_dtype = sinogram.dtype
    angles = np.asarray(angles)

    key = angles.astype(np.float64).tobytes()
    if _prog_cache.get("key") != key:
        units, groups = _build_tables(angles)
        gu, h, h_off, hcols, se_off, scols = _pack_tables(units)
        nc = _build_program(units, groups, hcols, scols, h_off, se_off)
        _prog_cache.update(key=key, units=units, groups=groups, gu=gu, h=h,
                           scols=scols, nc=nc)
    units = _prog_cache["units"]
    gu, h, scols, nc = (_prog_cache["gu"], _prog_cache["h"],
                        _prog_cache["scols"], _prog_cache["nc"])

    sino = sinogram.reshape(C, A, V, U).astype(np.float32)
    in_maps = [{"gu": gu, "h": h, "se": _build_se(sino[c], units, scols)}
               for c in range(C)]

    trace = bool(os.environ.get("BP_TRACE")) and _install_ntff_shim()
    res = run_bass_kernel_spmd(nc, in_maps, list(range(N_CORES)), trace=trace)
    _prog_cache["last_results"] = res
    vols = np.stack([
        _combine_flips(res.results[i]["out"].reshape(4, NZ, NT * FD))
        for i in range(N_CORES)
    ])
    return vols.reshape(B, C, NZ, NY, NX).astype(in_dtype, copy=False)


# revision 8
# speedup vs baseline: 4.8389x; 1.3864x over previous
import sys

sys.path.insert(0, "/opt/trn_rl_repo")
import numpy as np

# --- Problem geometry (hardcoded from the nn_DifferentiableBackprojection spec) ---
B, C, A, V, U = 1, 8, 120, 128, 128
NZ, NY, NX = 96, 96, 96
DSO, DSD, DU, DV, DVOX = 1000.0, 1500.0, 1.0, 1.0, 0.8
NYX = NY * NX
TY = TX = 3
TH, TW = NY // TY, NX // TX
FD = TH * TW  # 1024 cols per tile
NT = TY * TX
MARGIN = 1e-3
PACK = 128    # rows per matmul pack
N_CORES = 8

_prog_cache = {}


def _geom_jax(angles):
    """Bit-exact (vs reference) iu [A,NYX], mag [A,NYX] using jax f32 on CPU."""
    import jax
    import jax.numpy as jnp

    cpu = jax.devices("cpu")[0]

    @jax.jit
    def geom(angles):
        y = (jnp.arange(NY, dtype=jnp.float32) - (NY - 1) / 2.0) * DVOX
        x = (jnp.arange(NX, dtype=jnp.float32) - (NX - 1) / 2.0) * DVOX
        yg, xg = y[:, None], x[None, :]

        def one(ang):
            c, s = jnp.cos(ang), jnp.sin(ang)
            xr = xg * c + yg * s
            yr = -xg * s + yg * c
            mag = DSD / (DSO - xr)
            iu = (yr * mag / DU + (U - 1) / 2.0).reshape(NYX)
            return iu, mag.reshape(NYX)

        return jax.vmap(one)(angles)

    with jax.default_device(cpu):
        iu, mag = geom(jnp.asarray(angles, dtype=jnp.float32))
    return np.asarray(iu), np.asarray(mag)


def _find_groups(angles):
    """Group angles by mirror symmetry: [(canonical_idx, [(aidx, flip_j, urev)])]."""
    ang = np.asarray(angles, dtype=np.float64)
    n = len(ang)
    if not (n % 4 == 0 and np.allclose(ang, np.arange(n) * 2 * np.pi / n, atol=1e-5)):
        return [(i, [(i, 0, False)]) for i in range(n)]
    q = n // 4
    groups = []
    for a in range(q + 1):
        partners = [(a, 0, False)]
        seen = {a}
        for idx, j, urev in [((2 * q - a) % n, 1, True),
                             ((2 * q + a) % n, 2, False),
                             ((n - a) % n, 3, True)]:
            if idx not in seen:
                partners.append((idx, j, urev))
                seen.add(idx)
        groups.append((a, partners))
    allidx = sorted(i for _, ps in groups for i, _, _ in ps)
    assert allidx == list(range(n)), "angle symmetry grouping failed"
    return groups


def _tile_cols(t):
    ty, tx = t // TX, t % TX
    yy = np.arange(ty * TH, (ty + 1) * TH)
    xx = np.arange(tx * TW, (tx + 1) * TW)
    return (yy[:, None] * NX + xx[None, :]).reshape(-1)


def _build_tables(angles):
    """Per-(group,tile) packed tables.

    Per unit:
      gu   [U, FD] f16          u-hat * mag^2 * valid (shared by partners)
      L    list of (k, z)       nonzero h rows, z-major (len nl)
      hrow [nl, FD] f16         h values per L row
      rows [nl] int             sinogram v-row per L row
      npack = ceil(J*nl/128)
      pieces: per pack, list of (dst0, dst1, l0) sbuf-partition range ->
              L-range (wraps at partner boundaries), plus the partner ji
      P    [nseg, 128, NZ] f16  unpack matrices per piece
    """
    f32 = np.float32
    iu_all, mag_all = _geom_jax(angles)
    groups = _find_groups(angles)
    zg = ((np.arange(NZ, dtype=f32) - (NZ - 1) / 2.0) * DVOX).astype(f32)
    uu = np.arange(U, dtype=f32)

    units = []
    for ci, partners in groups:
        iu, mag = iu_all[ci], mag_all[ci]
        wv = (mag * mag) * ((iu >= 0) & (iu <= U - 1)).astype(f32)
        J = len(partners)
        for t in range(NT):
            cols = _tile_cols(t)
            iu_t, wv_t, mag_t = iu[cols], wv[cols], mag[cols]
            gu = (np.maximum(f32(0), f32(1) - np.abs(uu[:, None] - iu_t[None, :]))
                  * wv_t[None, :]).astype(np.float16)
            iv_t = zg[:, None] * mag_t[None, :] + f32((V - 1) / 2.0)  # [NZ, FD]
            lo = np.floor(iv_t.min(axis=1) - MARGIN).astype(np.int64)  # [NZ]
            hi = np.floor(iv_t.max(axis=1) + MARGIN).astype(np.int64) + 1
            L, hrow, vrow = [], [], []
            for z in range(NZ):
                for jr in range(lo[z], hi[z] + 1):
                    hv = np.maximum(f32(0), f32(1) - np.abs(iv_t[z] - f32(jr)))
                    if hv.max() <= 0:
                        continue
                    L.append((jr, z))
                    hrow.append(hv.astype(np.float16))
                    vrow.append(min(max(jr, 0), V - 1))
            nl = len(L)
            hrow = np.stack(hrow)                     # [nl, FD]
            vrow = np.array(vrow, np.int64)
            npack = -(-(J * nl) // PACK)
            # pieces: global row g = ji*nl + l ; pack p covers g in [128p,128p+128)
            pieces = []                               # per pack: list of segs
            for p in range(npack):
                segs = []
                g0, g1 = p * PACK, min((p + 1) * PACK, J * nl)
                g = g0
                while g < g1:
                    ji = g // nl
                    lend = min(g1, (ji + 1) * nl)
                    segs.append((g - g0, lend - g0, g - ji * nl, ji))
                    g = lend
                pieces.append(segs)
            nseg = sum(len(s) for s in pieces)
            P = np.zeros((nseg, PACK, NZ), np.float16)
            si = 0
            for p in range(npack):
                for (d0, d1, l0, ji) in pieces[p]:
                    for r in range(d0, d1):
                        P[si, r, L[l0 + r - d0][1]] = 1.0
                    si += 1
            units.append(dict(gu=gu, L=L, hrow=hrow, vrow=vrow, nl=nl, J=J,
                              npack=npack, pieces=pieces, P=P,
                              partners=partners, tile=t))
    return units, groups


def _pack_tables(units):
    """Flat dram arrays + offsets: gu [NU,U,FD]; h [HROWS,FD]; P [128, PSEGS*NZ]."""
    gu = np.stack([u["gu"] for u in units])
    h_off, p_off, se_off = [], [], []
    hr = pr = sc = 0
    for u in units:
        h_off.append(hr); p_off.append(pr); se_off.append(sc)
        hr += u["nl"]
        pr += u["P"].shape[0] * NZ
        sc += u["npack"] * PACK
    h = np.concatenate([u["hrow"] for u in units], axis=0)       # [HROWS, FD]
    # P packed partition-major: [128, total_segs*NZ]
    P = np.concatenate([np.transpose(u["P"], (1, 0, 2)).reshape(PACK, -1)
                        for u in units], axis=1)
    return gu, h, P, h_off, p_off, se_off, hr, pr, sc


def _build_se(sino_c, units, scols):
    """Packed gathered sinogram rows for one channel: [U, scols] f16."""
    se = np.zeros((U, scols), np.float16)
    col = 0
    for un in units:
        nl, vrow, J = un["nl"], un["vrow"], un["J"]
        blocks = []
        for (aidx, j, urev) in un["partners"]:
            g = sino_c[aidx][vrow]            # [nl, U]
            if urev:
                g = g[:, ::-1]
            blocks.append(g)
        blk = np.concatenate(blocks, axis=0)  # [J*nl, U]
        n = un["npack"] * PACK
        se[:, col:col + blk.shape[0]] = blk.T
        col += n
    assert col == scols
    return se


def _build_program(units, hrows, prows, scols, h_off, p_off, se_off):
    import concourse.bass as bass
    import concourse.tile as tile
    from concourse import mybir, bacc

    nu = len(units)
    nc = bacc.Bacc("TRN2", target_bir_lowering=False, debug=False)
    gu_d = nc.dram_tensor("gu", (nu, U, FD), mybir.dt.float16, kind="ExternalInput")
    h_d = nc.dram_tensor("h", (hrows, FD), mybir.dt.float16, kind="ExternalInput")
    p_d = nc.dram_tensor("P", (PACK, prows), mybir.dt.float16, kind="ExternalInput")
    se_d = nc.dram_tensor("se", (U, scols), mybir.dt.float16, kind="ExternalInput")
    out_d = nc.dram_tensor("out", (4, NZ, NT * FD), mybir.dt.float32,
                           kind="ExternalOutput")

    with tile.TileContext(nc) as tc:
        with (
            tc.tile_pool(name="persist", bufs=1) as pp,
            tc.tile_pool(name="io", bufs=2) as io,
            tc.tile_pool(name="wk", bufs=3) as wk,
            tc.tile_pool(name="ps", bufs=2, space=bass.MemorySpace.PSUM) as ps,
            tc.tile_pool(name="psa", bufs=2, space=bass.MemorySpace.PSUM) as psa,
        ):
            acc = []
            for j in range(4):
                a = pp.tile([NZ, NT * FD], mybir.dt.float32, name=f"acc{j}")
                nc.vector.memset(a[:], 0.0)
                acc.append(a)

            # iterate tiles outer so acc columns stream nicely; groups inner
            ngroups = len(units) // NT
            for t in range(NT):
                for gi in range(ngroups):
                    ui = gi * NT + t
                    un = units[ui]
                    nl, J, npack = un["nl"], un["J"], un["npack"]
                    pieces = un["pieces"]
                    gu_t = io.tile([U, FD], mybir.dt.float16, tag="gu")
                    nc.sync.dma_start(gu_t[:], gu_d.ap()[ui])
                    se_t = io.tile([U, npack * PACK], mybir.dt.float16, tag="se")
                    nc.sync.dma_start(
                        se_t[:], se_d.ap()[:, se_off[ui]:se_off[ui] + npack * PACK])
                    nseg = un["P"].shape[0]
                    pseg_t = io.tile([PACK, nseg * NZ], mybir.dt.float16,
                                     tag="pseg")
                    nc.sync.dma_start(
                        pseg_t[:], p_d.ap()[:, p_off[ui]:p_off[ui] + nseg * NZ])

                    # h tiles per pack: 1-2 piece DMAs each (h shared across j),
                    # pad rows backfilled from the unit's own h to avoid NaNs
                    h_tiles = []
                    for p in range(npack):
                        ht = io.tile([PACK, FD], mybir.dt.float16, tag=f"h{p % 3}")
                        end = 0
                        for (d0, d1, l0, ji) in pieces[p]:
                            nc.sync.dma_start(
                                ht[d0:d1, :],
                                h_d.ap()[h_off[ui] + l0:h_off[ui] + l0 + (d1 - d0)])
                            end = d1
                        if end < PACK:
                            nc.sync.dma_start(
                                ht[end:PACK, :],
                                h_d.ap()[h_off[ui]:h_off[ui] + (PACK - end)])
                        h_tiles.append(ht)

                    # per-partner PSUM accumulators, evacuated when finished
                    accp = {}
                    si = 0
                    for p in range(npack):
                        te = ps.tile([PACK, FD], mybir.dt.float32, tag="te")
                        sek = se_t[:, p * PACK:(p + 1) * PACK]
                        for h2 in range(0, FD, 512):
                            nc.tensor.matmul(te[:, h2:h2 + 512], sek,
                                             gu_t[:, h2:h2 + 512],
                                             start=True, stop=True)
                        m16 = wk.tile([PACK, FD], mybir.dt.float16, tag="m16")
                        nc.scalar.copy(m16[:], te[:])
                        nc.vector.tensor_mul(m16[:], m16[:], h_tiles[p][:])
                        for (d0, d1, l0, ji) in pieces[p]:
                            if ji not in accp:
                                accp[ji] = [psa.tile([NZ, FD], mybir.dt.float32,
                                                     tag="accp", name=f"ap{ui}_{ji}"),
                                            True]
                            ap, first = accp[ji]
                            last = l0 + (d1 - d0) == nl
                            Pm = pseg_t[:, si * NZ:(si + 1) * NZ]
                            for h2 in range(0, FD, 512):
                                nc.tensor.matmul(ap[:, h2:h2 + 512], Pm,
                                                 m16[:, h2:h2 + 512],
                                                 start=first, stop=last)
                            accp[ji][1] = False
                            si += 1
                            # if this partner's rows end within this pack, evacuate
                            if last:
                                at = acc[un["partners"][ji][1]][:, t * FD:(t + 1) * FD]
                                nc.vector.tensor_add(at, at, ap[:])
                                del accp[ji]
                    assert not accp, f"unit {ui}: unfinished partners {accp.keys()}"
            for j in range(4):
                nc.sync.dma_start(out_d.ap()[j], acc[j][:])
    nc.compile()
    return nc


def _install_ntff_shim():
    """Provide antenv.axon_hooks (missing in this image) so trace=True works."""
    import types, importlib

    try:
        from antenv.axon_hooks import get_axon_ntff_profile_hook  # noqa: F401

        return True
    except ImportError:
        pass
    try:
        import antenv

        mod = types.ModuleType("antenv.axon_hooks")
        mod._hook = None

        def set_axon_ntff_profile_hook(h):
            mod._hook = h

        def get_axon_ntff_profile_hook():
            return mod._hook

        mod.set_axon_ntff_profile_hook = set_axon_ntff_profile_hook
        mod.get_axon_ntff_profile_hook = get_axon_ntff_profile_hook
        sys.modules["antenv.axon_hooks"] = mod
        antenv.axon_hooks = mod
        if "/root/.axon_site" not in sys.path:
            sys.path.insert(0, "/root/.axon_site")
        boot = importlib.import_module("trn_agent_boot.trn_boot")
        hook = boot._ntff_profile_via_ctypes("/opt/axon/libaxon_pjrt.so")
        if hook is None:
            return False
        mod._hook = hook
        return True
    except Exception as e:  # pragma: no cover
        print(f"ntff shim failed: {e}")
        return False


def _combine_flips(acc_f32):
    """acc [4, NZ, NT*FD] tile-major -> volume [NZ, NY, NX]."""
    full = np.empty((4, NZ, NY, NX), dtype=np.float32)
    v = acc_f32.reshape(4, NZ, TY, TX, TH, TW)
    for ty in range(TY):
        for tx in range(TX):
            full[:, :, ty * TH:(ty + 1) * TH, tx * TW:(tx + 1) * TW] = v[:, :, ty, tx]
    out = full[0]
    out += full[1][:, :, ::-1]
    out += full[2][:, ::-1, ::-1]
    out += full[3][:, ::-1, :]
    return out


def kernel(sinogram, angles):
    import os
    from concourse.bass_utils import run_bass_kernel_spmd

    sinogram = np.asarray(sinogram)
    in_dtype = sinogram.dtype
    angles = np.asarray(angles)

    key = angles.astype(np.float64).tobytes()
    if _prog_cache.get("key") != key:
        units, groups = _build_tables(angles)
        gu, h, P, h_off, p_off, se_off, hrows, prows, scols = _pack_tables(units)
        nc = _build_program(units, hrows, prows, scols, h_off, p_off, se_off)
        _prog_cache.update(key=key, units=units, gu=gu, h=h, P=P,
                           scols=scols, nc=nc)
    units = _prog_cache["units"]
    gu, h, P, scols, nc = (_prog_cache["gu"], _prog_cache["h"], _prog_cache["P"],
                           _prog_cache["scols"], _prog_cache["nc"])

    sino = sinogram.reshape(C, A, V, U).astype(np.float32)
    in_maps = [{"gu": gu, "h": h, "P": P, "se": _build_se(sino[c], units, scols)}
               for c in range(C)]

    trace = bool(os.environ.get("BP_TRACE")) and _install_ntff_shim()
    res = run_bass_kernel_spmd(nc, in_maps, list(range(N_CORES)), trace=trace)
    _prog_cache["last_results"] = res
    vols = np.stack([
        _combine_flips(res.results[i]["out"].reshape(4, NZ, NT * FD))
        for i in range(N_CORES)
    ])
    return vols.reshape(B, C, NZ, NY, NX).astype(in_dtype, copy=False)
